# revision 8
# baseline (speedup 1.0000x reference)
"""Trainium2 Bass kernel for nn_AttentionBlock (B=4, S=2048, D=1024, H=16, Dh=64).

Sharding: 8 cores = 4 batches x 2 head-groups (8 heads each). Every core runs
the same Bass program on different input slices. The output projection is
row-sharded over head-groups, so the host sums the two partial outputs per
batch (the "all-reduce" of the sharding hint, done on host since we return
full outputs anyway).

Per-core pipeline (all matmuls fp32r = full-rate fp32):
  A) QKV projection: lhsT = X^T chunks [128,128], rhs = Wqkv [128,1536 cols]
     -> psum [128(S-tile), 512] per q/k/v. RoPE applied on DVE in
     [S, (h,Dh)] layout, then PE-transposed per head into qT/kT [Dh, S]
     packs. V goes to SBUF augmented with a ones column (V_aug [Sk,65]).
  B) Attention per head: scores computed PRE-TRANSPOSED
     sT[Sk-tile 128, Sq 512] = kT_tile.T @ qT_group. Causal mask added via a
     PE matmul with (-1e30*I) @ mask01 constants. exp on ScalarE
     (PSUM->SBUF). AV: x_aug^T[65, Sq] += V_aug_tile.T @ pT, where row 64
     accumulates the softmax denominator Z for free. Normalization happens
     after a small PE transpose (reciprocal + per-partition scale), then a
     transpose back into xT packs for the output projection.
  C) Output projection: out[Sq,512] += xT_pair.T @ WoutPair, DMA to HBM.
"""

import sys

for _p in ("/opt/pypackages", "/opt/trn_rl_repo"):
    if _p not in sys.path:
        sys.path.insert(0, _p)

import numpy as np

B, S, D, H, Dh = 4, 2048, 1024, 16, 64
HL = H // 2          # heads per core
NCORES = 8
ST = S // 128        # 16 S-tiles of 128
NG = S // 512        # 4 q-groups of 512
MAX_WAVELENGTH = 10000.0

_CACHE = {}


def _build_bass():
    import concourse.bass as bass
    import concourse.mybir as mybir
    from concourse import bacc
    from concourse.tile import TileContext
    from contextlib import ExitStack

    f32 = mybir.dt.float32
    f32r = mybir.dt.float32r
    AT = mybir.ActivationFunctionType
    OP = mybir.AluOpType

    nc = bacc.Bacc("TRN2", target_bir_lowering=False)

    xt_d = nc.dram_tensor("xt", [D, S], f32r, kind="ExternalInput")
    wqkv_d = nc.dram_tensor("wqkv", [D, 3 * HL * Dh], f32r, kind="ExternalInput")
    wout_d = nc.dram_tensor("woutp", [4, 128, D], f32r, kind="ExternalInput")
    cos_d = nc.dram_tensor("cost", [S, Dh // 2], f32, kind="ExternalInput")
    sin_d = nc.dram_tensor("sint", [S, Dh // 2], f32, kind="ExternalInput")
    negid_d = nc.dram_tensor("negid", [128, 128], f32r, kind="ExternalInput")
    maska_d = nc.dram_tensor("maska", [4, 128, 512], f32r, kind="ExternalInput")
    ident_d = nc.dram_tensor("ident", [128, 128], f32, kind="ExternalInput")
    out_d = nc.dram_tensor("out", [S, D], f32, kind="ExternalOutput")

    def r(ap):
        return ap

    with TileContext(nc) as tc, ExitStack() as ctx:
        consts = ctx.enter_context(tc.tile_pool(name="consts", bufs=1))
        persist = ctx.enter_context(tc.tile_pool(name="persist", bufs=1))

        ident_sb = consts.tile([128, 128], f32, tag="ident")
        nc.sync.dma_start(ident_sb, ident_d[:, :])
        cos_sb = consts.tile([128, ST, 32], f32, tag="cos")
        nc.sync.dma_start(cos_sb, cos_d.rearrange("(t p) f -> p t f", p=128))
        sin_sb = consts.tile([128, ST, 32], f32, tag="sin")
        nc.sync.dma_start(sin_sb, sin_d.rearrange("(t p) f -> p t f", p=128))

        # qT/kT/xT packs: pair slot j holds head 2j on partitions 0:64 and
        # head 2j+1 on partitions 64:128.
        qT = persist.tile([128, 4, S], f32r, tag="qT")
        kT = persist.tile([128, 4, S], f32r, tag="kT")
        vaug = persist.tile([128, HL, ST, Dh + 1], f32r, tag="vaug")
        nc.scalar.activation(
            vaug[:, :, :, Dh : Dh + 1], vaug[:, :, :, Dh : Dh + 1],
            AT.Identity, bias=1.0, scale=0.0,
        )

        # ---------------- Phase A: QKV projection + RoPE + transposes ------
        with tc.tile_pool(name="wqp", bufs=1) as wq_pool, \
             tc.tile_pool(name="xin", bufs=2) as xin_pool, \
             tc.tile_pool(name="ropew", bufs=3) as rw_pool, \
             tc.tile_pool(name="psA", bufs=2, space="PSUM") as psA, \
             tc.tile_pool(name="psT", bufs=2, space="PSUM") as psT:
            wq_sb = wq_pool.tile([128, 8, 1536], f32r, tag="wqkv")
            nc.sync.dma_start(wq_sb, wqkv_d.rearrange("(c p) n -> p c n", p=128))
            for si in range(ST):
                xt_t = xin_pool.tile([128, 8, 128], f32r, tag="xt")
                nc.sync.dma_start(
                    xt_t,
                    xt_d[:, si * 128 : (si + 1) * 128].rearrange(
                        "(c p) s -> p c s", p=128
                    ),
                )
                ps_q = psA.tile([128, 512], f32, tag="psq")
                ps_k = psA.tile([128, 512], f32, tag="psk")
                ps_v = psA.tile([128, 512], f32, tag="psv")
                for c in range(8):
                    lt = r(xt_t[:, c, :])
                    nc.tensor.matmul(ps_q, lt, r(wq_sb[:, c, 0:512]),
                                     start=(c == 0), stop=(c == 7))
                    nc.tensor.matmul(ps_k, lt, r(wq_sb[:, c, 512:1024]),
                                     start=(c == 0), stop=(c == 7))
                    nc.tensor.matmul(ps_v, lt, r(wq_sb[:, c, 1024:1536]),
                                     start=(c == 0), stop=(c == 7))
                # V -> vaug (heads interleaved in psum free dim)
                nc.scalar.copy(
                    vaug[:, :, si, 0:Dh],
                    ps_v.rearrange("p (h d) -> p h d", h=HL),
                )
                cos_b = cos_sb[:, si, None, :].to_broadcast((128, HL, 32))
                sin_b = sin_sb[:, si, None, :].to_broadcast((128, HL, 32))
                for ps, dstT in ((ps_q, qT), (ps_k, kT)):
                    v3 = ps.rearrange("p (h d) -> p h d", h=HL)
                    x1, x2 = v3[:, :, 0:32], v3[:, :, 32:64]
                    rot = rw_pool.tile([128, HL, Dh], f32, tag="rot")
                    t1 = rw_pool.tile([128, HL, 32], f32, tag="t1")
                    t2 = rw_pool.tile([128, HL, 32], f32, tag="t2")
                    nc.vector.tensor_tensor(t1, x1, cos_b, OP.mult)
                    nc.vector.tensor_tensor(t2, x2, sin_b, OP.mult)
                    nc.vector.tensor_tensor(rot[:, :, 0:32], t1, t2, OP.subtract)
                    nc.vector.tensor_tensor(t1, x1, sin_b, OP.mult)
                    nc.vector.tensor_tensor(t2, x2, cos_b, OP.mult)
                    nc.vector.tensor_tensor(rot[:, :, 32:64], t1, t2, OP.add)
                    for h in range(HL):
                        ps_t = psT.tile([64, 128], f32, tag="pst")
                        nc.tensor.transpose(ps_t, rot[:, h, :], ident_sb)
                        hp, hh = h % 2, h // 2
                        nc.scalar.copy(
                            dstT[64 * hp : 64 * hp + 64, hh,
                                 si * 128 : (si + 1) * 128],
                            ps_t,
                        )

        # ---------------- Phase B: attention per head ----------------------
        bc = ctx.enter_context(tc.tile_pool(name="bc", bufs=1))
        xT = bc.tile([128, 4, S], f32r, tag="xT")
        negid_sb = bc.tile([128, 128], f32r, tag="negid")
        nc.sync.dma_start(negid_sb, negid_d[:, :])
        maska_sb = bc.tile([128, 4, 512], f32r, tag="maska")
        nc.sync.dma_start(maska_sb, maska_d.rearrange("v p n -> p v n"))
        wout_sb = bc.tile([128, 4, 1024], f32r, tag="wout")
        nc.sync.dma_start(wout_sb, wout_d.rearrange("q p n -> p q n"))
        with tc.tile_pool(name="ptp", bufs=6) as pt_pool, \
             tc.tile_pool(name="nrm", bufs=3) as nrm_pool, \
             tc.tile_pool(name="psS", bufs=3, space="PSUM") as psS, \
             tc.tile_pool(name="psX", bufs=2, space="PSUM") as psX, \
             tc.tile_pool(name="psN", bufs=1, space="PSUM") as psN:
            for h in range(HL):
                hp, hh = h % 2, h // 2
                kslice = kT[64 * hp : 64 * hp + 64, hh, :]
                qslice = qT[64 * hp : 64 * hp + 64, hh, :]
                for g in range(NG):
                    ps_x = psX.tile([Dh + 1, 512], f32, tag="psx")
                    nj = 4 * (g + 1)
                    qg = r(qslice[:, g * 512 : (g + 1) * 512])
                    for j in range(nj):
                        ps_s = psS.tile([128, 512], f32, tag="pss")
                        diag = j >= 4 * g
                        nc.tensor.matmul(
                            ps_s,
                            r(kslice[:, j * 128 : (j + 1) * 128]),
                            qg,
                            start=True,
                            stop=not diag,
                        )
                        if diag:
                            v = j - 4 * g
                            nc.tensor.matmul(
                                ps_s, r(negid_sb), r(maska_sb[:, v, :]),
                                start=False, stop=True,
                            )
                        pt = pt_pool.tile([128, 512], f32r, tag="pt")
                        nc.scalar.activation(pt, ps_s, AT.Exp)
                        nc.tensor.matmul(
                            ps_x, r(vaug[:, h, j, :]), r(pt),
                            start=(j == 0), stop=(j == nj - 1),
                        )
                    # normalize x_aug^T [65, 512] and store into xT pack
                    xa_sb = nrm_pool.tile([Dh + 1, 512], f32, tag="xa")
                    nc.scalar.copy(xa_sb, ps_x)
                    for m in range(4):
                        ps_xc = psN.tile([128, Dh + 1], f32, tag="psxc")
                        nc.tensor.transpose(
                            ps_xc,
                            xa_sb[:, m * 128 : (m + 1) * 128],
                            ident_sb[0 : Dh + 1, 0 : Dh + 1],
                        )
                        rcp = nrm_pool.tile([128, 1], f32, tag="rcp")
                        nc.vector.reciprocal(rcp, ps_xc[:, Dh : Dh + 1])
                        xn = nrm_pool.tile([128, Dh], f32, tag="xn")
                        nc.vector.tensor_scalar_mul(xn, ps_xc[:, 0:Dh], rcp)
                        ps_xt = psN.tile([64, 128], f32, tag="psxt")
                        nc.tensor.transpose(ps_xt, xn, ident_sb)
                        q0 = g * 512 + m * 128
                        nc.scalar.copy(
                            xT[64 * hp : 64 * hp + 64, hh, q0 : q0 + 128],
                            ps_xt,
                        )

        # ---------------- Phase C: output projection -----------------------
        with tc.tile_pool(name="outp", bufs=3) as out_pool, \
             tc.tile_pool(name="psO", bufs=2, space="PSUM") as psO:
            for m in range(ST):
                for half in range(2):
                    ps_o = psO.tile([128, 512], f32, tag="pso")
                    for p in range(4):
                        nc.tensor.matmul(
                            ps_o,
                            r(xT[:, p, m * 128 : (m + 1) * 128]),
                            r(wout_sb[:, p, half * 512 : (half + 1) * 512]),
                            start=(p == 0),
                            stop=(p == 3),
                        )
                    ob = out_pool.tile([128, 512], f32, tag="ob")
                    nc.scalar.copy(ob, ps_o)
                    nc.sync.dma_start(
                        out_d[m * 128 : (m + 1) * 128,
                              half * 512 : (half + 1) * 512],
                        ob,
                    )

    nc.compile()
    return nc


def _numpy_fallback(x, w_q, w_k, w_v, w_out, seg, mask):
    """Exact numpy replica of the reference for non-causal masks."""
    frac = (2.0 * np.arange(Dh // 2, dtype=np.float32)) / Dh
    ts = (MAX_WAVELENGTH ** frac).astype(np.float32)

    def rope(t, pos):
        sinu = pos.astype(np.float32)[:, :, None] / ts  # [B,S,32]
        sn, cs = np.sin(sinu), np.cos(sinu)
        sn, cs = sn[:, :, None, :], cs[:, :, None, :]
        f, s_ = t[..., :32], t[..., 32:]
        return np.concatenate([f * cs - s_ * sn, s_ * cs + f * sn], -1)

    q = np.einsum("bsd,dhk->bshk", x, w_q)
    k = np.einsum("bsd,dhk->bshk", x, w_k)
    v = np.einsum("bsd,dhk->bshk", x, w_v)
    q, k = rope(q, seg), rope(k, seg)
    q = q / np.sqrt(np.float32(Dh))
    attn = np.einsum("bqhd,bkhd->bhqk", q, k)
    attn = np.where(mask, attn, np.finfo(np.float32).min)
    attn = attn - attn.max(-1, keepdims=True)
    e = np.exp(attn)
    attn = e / e.sum(-1, keepdims=True)
    xo = np.einsum("bhqk,bkhd->bqhd", attn, v)
    return np.einsum("bqhd,hdm->bqm", xo, w_out).astype(np.float32)


def _host_inputs(x, w_q, w_k, w_v, w_out, seg):
    frac = (2.0 * np.arange(Dh // 2, dtype=np.float32)) / Dh
    ts = (MAX_WAVELENGTH ** frac).astype(np.float32)
    negid = (np.eye(128, dtype=np.float32) * -1e30).astype(np.float32)
    ident = np.eye(128, dtype=np.float32)
    rr = np.arange(128)[:, None]
    cc = np.arange(512)[None, :]
    maska = np.stack(
        [(rr + 128 * v > cc).astype(np.float32) for v in range(4)]
    )  # [4,128,512], 1 where masked

    in_maps = []
    for core in range(NCORES):
        b, g = core // 2, core % 2
        hs = slice(g * HL, (g + 1) * HL)
        wq_s = (w_q[:, hs, :] / np.float32(np.sqrt(Dh))).reshape(D, HL * Dh)
        wk_s = w_k[:, hs, :].reshape(D, HL * Dh)
        wv_s = w_v[:, hs, :].reshape(D, HL * Dh)
        wqkv = np.ascontiguousarray(
            np.concatenate([wq_s, wk_s, wv_s], axis=1), dtype=np.float32
        )
        woutp = np.stack(
            [
                w_out[g * HL + 2 * p : g * HL + 2 * p + 2].reshape(128, D)
                for p in range(4)
            ]
        ).astype(np.float32)
        sinu = seg[b].astype(np.float32)[:, None] / ts  # [S, 32]
        in_maps.append(
            {
                "xt": np.ascontiguousarray(x[b].T, dtype=np.float32),
                "wqkv": wqkv,
                "woutp": np.ascontiguousarray(woutp),
                "cost": np.cos(sinu).astype(np.float32),
                "sint": np.sin(sinu).astype(np.float32),
                "negid": negid,
                "maska": np.ascontiguousarray(maska),
                "ident": ident,
            }
        )
    return in_maps


def _run(in_maps, trace=False):
    from concourse.bass_utils import run_bass_kernel_spmd

    if "nc" not in _CACHE:
        _CACHE["nc"] = _build_bass()
    return run_bass_kernel_spmd(
        _CACHE["nc"], in_maps, core_ids=list(range(NCORES)), trace=trace
    )


def kernel(**inputs):
    x = np.asarray(inputs["inputs"], dtype=np.float32)
    w_q = np.asarray(inputs["w_q"], dtype=np.float32)
    w_k = np.asarray(inputs["w_k"], dtype=np.float32)
    w_v = np.asarray(inputs["w_v"], dtype=np.float32)
    w_out = np.asarray(inputs["w_out"], dtype=np.float32)
    seg = np.asarray(inputs["segment_positions"])
    mask = np.asarray(inputs["mask"])

    causal = np.tril(np.ones((S, S), dtype=bool))
    if not all(np.array_equal(mask[b, 0], causal) for b in range(B)):
        return _numpy_fallback(x, w_q, w_k, w_v, w_out, seg, mask)

    in_maps = _host_inputs(x, w_q, w_k, w_v, w_out, seg)
    res = _run(in_maps)
    outs = [r_["out"] for r_ in res.results]
    result = np.empty((B, S, D), dtype=np.float32)
    for b in range(B):
        result[b] = outs[2 * b] + outs[2 * b + 1]
    return result
